# revision 22
# baseline (speedup 1.0000x reference)
"""Fused MLA preprocess kernel for Trainium2, 8 NeuronCores, token-data-parallel.

Layout strategy: every on-device tensor is kept "transposed" (feature on the
SBUF partition dim, tokens on the free dim) so that all three matmuls contract
over the partition dim with weights in their natural layout and zero on-device
transposes.  The host does the pure data-movement parts: sharding tokens,
pre-transposing/blocking inputs, re-transposing outputs, and the cache scatter.

Matmul dtype is float32r (fp32 with ~12-bit mantissa, full PE rate); inputs are
pre-rounded on the host so HW and any simulation agree bit-wise.
"""

import contextlib
import os
import sys

sys.path.insert(0, "/opt/trn_rl_repo")
os.environ.setdefault("MYCRO_LOCAL_CACHE", "1")

import numpy as np

import concourse.bacc as bacc
import concourse.bass as bass
import concourse.mybir as mybir
import concourse.tile as tile

# ---------------------------------------------------------------- constants
T, H = 4096, 7168
Q_LORA, KV_LORA = 1536, 512
ROPE, NOPE, NH = 64, 128, 16
QK = NOPE + ROPE          # 192
SLOTS = 16384
EPS = 1e-6
ROPE_END = Q_LORA + KV_LORA + ROPE   # 2112

NCORES = 8
TC = T // NCORES          # 512 tokens per core
P = 128
KO1 = H // P              # 56 k-tiles for the qkv_a matmul
KQ = 8                    # hidden split into 8 quarters of 7 k-tiles
KJ = KO1 // KQ            # 7
TQ = Q_LORA // P          # 12 q-lora k-tiles
TKV = KV_LORA // P        # 4
M1 = 17                   # step-1 m-tiles: 12 q + 4 latent + 1 rope(64)
NPAIR = NH // 2           # 8 packed rope pairs

F32 = mybir.dt.float32
F32R = mybir.dt.float32r

AluOp = mybir.AluOpType
Act = mybir.ActivationFunctionType


def _round_f32r(x: np.ndarray) -> np.ndarray:
    """Round-to-nearest fp32 -> fp32r (12 low mantissa bits cleared)."""
    u = np.ascontiguousarray(x, dtype=np.float32).view(np.uint32)
    r = (u + np.uint32(0x7FF) + ((u >> np.uint32(12)) & np.uint32(1))) & np.uint32(
        0xFFFFF000
    )
    return r.view(np.float32)


# ---------------------------------------------------------------- program
def build_program(loop_n: int | None = None) -> bass.Bass:
    nc = bacc.Bacc("TRN2", target_bir_lowering=False, debug=False)

    # Round-robin DMA issue between the two HWDGE-capable engines (SP, ACT)
    # so neither sequencer's per-dma_start issue overhead serializes the
    # kernel.  Weight/activation loads lean on SP; ACT takes every third.
    _dma_ctr = [0]

    def dma(out_ap, in_ap):
        _dma_ctr[0] += 1
        eng = nc.scalar if (_dma_ctr[0] % 3 == 0) else nc.sync
        return eng.dma_start(out_ap, in_ap)

    hid = nc.dram_tensor("hid", (KQ, P, KJ, TC), F32R, kind="ExternalInput")
    w1f = nc.dram_tensor("w1f", (16, KQ, P, KJ, P), F32R, kind="ExternalInput")
    w1r = nc.dram_tensor("w1r", (KQ, P, KJ, ROPE), F32R, kind="ExternalInput")
    w2n = nc.dram_tensor("w2n", (NH, P, TQ, P), F32R, kind="ExternalInput")
    w2p = nc.dram_tensor("w2p", (NPAIR, P, TQ, P), F32R, kind="ExternalInput")
    wkc = nc.dram_tensor("wkc", (NH, P, KV_LORA), F32R, kind="ExternalInput")
    lnq = nc.dram_tensor("lnq", (P, TQ), F32, kind="ExternalInput")
    lnkv = nc.dram_tensor("lnkv", (P, TKV), F32, kind="ExternalInput")
    cos4 = nc.dram_tensor("cos4", (P, TC), F32, kind="ExternalInput")
    sin4 = nc.dram_tensor("sin4", (P, TC), F32, kind="ExternalInput")
    ones_d = nc.dram_tensor("ones_d", (P, 1), F32R, kind="ExternalInput")
    onesm_d = nc.dram_tensor("onesm_d", (1, P), F32R, kind="ExternalInput")
    qoutT = nc.dram_tensor("qoutT", (NH, KV_LORA + ROPE, TC), F32, kind="ExternalOutput")
    koutT = nc.dram_tensor("koutT", (KV_LORA + ROPE, TC), F32, kind="ExternalOutput")

    with tile.TileContext(nc) as tc:
        with (
            tc.tile_pool(name="hid_sb", bufs=1) as hid_pool,
            tc.tile_pool(name="w1_sb", bufs=3) as w1_pool,
            tc.tile_pool(name="qkv_sb", bufs=1) as qkv_pool,
            tc.tile_pool(name="sq_sb", bufs=1) as sq_pool,
            tc.tile_pool(name="w2_sb", bufs=2) as w2_pool,
            tc.tile_pool(name="wkc_sb", bufs=2) as wkc_pool,
            tc.tile_pool(name="const_sb", bufs=1) as const_pool,
            tc.tile_pool(name="qn_sb", bufs=2) as qn_pool,
            tc.tile_pool(name="st_sb", bufs=4) as st_pool,
            tc.tile_pool(name="tmp_sb", bufs=1) as tmp_pool,
            tc.tile_pool(name="ps1", bufs=2, space="PSUM") as ps1_pool,
            tc.tile_pool(name="pss", bufs=1, space="PSUM") as pss_pool,
            tc.tile_pool(name="psb", bufs=1, space="PSUM") as psb_pool,
            tc.tile_pool(name="ps23", bufs=3, space="PSUM") as ps23_pool,
            (
                tc.For_i(0, loop_n, 1)
                if loop_n is not None
                else contextlib.nullcontext()
            ),
        ):
            # ---- constants / small inputs
            lnq_t = const_pool.tile([P, TQ], F32, tag="lnq")
            lnkv_t = const_pool.tile([P, TKV], F32, tag="lnkv")
            cos_t = const_pool.tile([P, TC], F32, tag="cos4")
            sin_t = const_pool.tile([P, TC], F32, tag="sin4")
            ones_k = const_pool.tile([P, 1], F32R, tag="ones_k")   # lhsT for column sums
            ones_m = const_pool.tile([1, P], F32R, tag="ones_m")   # lhsT for broadcast
            s_q = const_pool.tile([1, TC], F32R, tag="s_q")
            s_kv = const_pool.tile([1, TC], F32R, tag="s_kv")
            ms_q = const_pool.tile([1, TC], F32, tag="ms_q")
            ms_kv = const_pool.tile([1, TC], F32, tag="ms_kv")
            dma(lnq_t[:], lnq[:])
            dma(lnkv_t[:], lnkv[:])
            dma(cos_t[:], cos4[:])
            dma(sin_t[:], sin4[:])
            dma(ones_k[:], ones_d[:])
            dma(ones_m[:], onesm_d[:])

            # ---- resident hidden (transposed, blocked by k-quarter)
            hid_tiles = []
            for kq in range(KQ):
                ht = hid_pool.tile([P, KJ, TC], F32R, tag=f"hid{kq}")
                dma(ht[:], hid[kq])
                hid_tiles.append(ht)

            # ---- step 1: qkvT = w_qkv_a.T @ hiddenT   (17 m-tiles)
            qkv_tiles = []          # 12 q (f32r) + 4 latent (f32); rope stays in PSUM
            sumq_ps = pss_pool.tile([1, TC], F32, tag="sums")
            sumkv_ps = None
            sq_bq = None
            sq_bkv = None
            rope_ps = None

            for mt in range(M1):
                msz = P if mt < 16 else ROPE
                ps = ps1_pool.tile([msz, TC], F32, tag="ps1")
                for kq in range(KQ):
                    wt = w1_pool.tile([P, KJ, msz], F32R, tag="w1")
                    if mt < 16:
                        dma(wt[:], w1f[mt, kq])
                    else:
                        dma(wt[:], w1r[kq])
                    for kj in range(KJ):
                        nc.tensor.matmul(
                            ps[:],
                            wt[:, kj, :],
                            hid_tiles[kq][:, kj, :],
                            start=(kq == 0 and kj == 0),
                            stop=(kq == KQ - 1 and kj == KJ - 1),
                        )

                if mt < TQ:
                    # q-lora tile: raw copyback (f32r) + squares for rmsnorm
                    qt = qkv_pool.tile([P, TC], F32R, tag=f"qkv{mt}")
                    nc.vector.tensor_copy(qt[:], ps[:])
                    sq = sq_pool.tile([P, TC], F32R, tag="sq")
                    nc.scalar.activation(sq[:], ps[:], Act.Square)
                    nc.tensor.matmul(
                        sumq_ps[:], ones_k[:], sq[:],
                        start=(mt == 0), stop=(mt == TQ - 1),
                    )
                    qkv_tiles.append(qt)
                elif mt < 16:
                    # kv-latent tile: raw copyback (f32) + squares
                    qt = qkv_pool.tile([P, TC], F32, tag=f"qkv{mt}")
                    nc.vector.tensor_copy(qt[:], ps[:])
                    sq = sq_pool.tile([P, TC], F32R, tag="sq")
                    nc.scalar.activation(sq[:], ps[:], Act.Square)
                    if mt == TQ:
                        sumkv_ps = pss_pool.tile([1, TC], F32, tag="sums")
                    nc.tensor.matmul(
                        sumkv_ps[:], ones_k[:], sq[:],
                        start=(mt == TQ), stop=(mt == 15),
                    )
                    qkv_tiles.append(qt)
                else:
                    rope_ps = ps  # kept in PSUM for the k-rope below

                if mt == TQ - 1:
                    # s_q = 1/sqrt(mean + eps), broadcast to 128 partitions
                    nc.scalar.activation(
                        ms_q[:], sumq_ps[:], Act.Copy, scale=1.0 / Q_LORA, bias=EPS
                    )
                    nc.vector.reciprocal(ms_q[:], ms_q[:])
                    nc.scalar.activation(s_q[:], ms_q[:], Act.Sqrt)
                    sq_bq = psb_pool.tile([P, TC], F32, tag="bq")
                    nc.tensor.matmul(sq_bq[:], ones_m[:], s_q[:], start=True, stop=True)
                    # scale all 12 q tiles in place: q = (q * lnw) * s
                    for j in range(TQ):
                        nc.vector.scalar_tensor_tensor(
                            qkv_tiles[j][:], qkv_tiles[j][:], lnq_t[:, j : j + 1],
                            sq_bq[:], AluOp.mult, AluOp.mult,
                        )
                if mt == 15:
                    nc.scalar.activation(
                        ms_kv[:], sumkv_ps[:], Act.Copy, scale=1.0 / KV_LORA, bias=EPS
                    )
                    nc.vector.reciprocal(ms_kv[:], ms_kv[:])
                    nc.scalar.activation(s_kv[:], ms_kv[:], Act.Sqrt)
                    sq_bkv = psb_pool.tile([P, TC], F32, tag="bkv")
                    nc.tensor.matmul(sq_bkv[:], ones_m[:], s_kv[:], start=True, stop=True)
                    # k_nope tiles -> staging -> DRAM
                    for j in range(TKV):
                        st = st_pool.tile([P, TC], F32, tag="st")
                        nc.vector.scalar_tensor_tensor(
                            st[:], qkv_tiles[TQ + j][:], lnkv_t[:, j : j + 1],
                            sq_bkv[:], AluOp.mult, AluOp.mult,
                        )
                        dma(koutT[j * P : (j + 1) * P, :], st[:])

            # ---- k-rope (rows live in rope_ps [64, TC]).  sin_t carries
            # alternating signs per 32-block, so after a partition-swap of the
            # halves the result is just P1 + swap(P2):
            #   out[0:32] = x1*cos - x2*sin;  out[32:64] = x2*cos + x1*sin
            st = st_pool.tile([ROPE, TC], F32, tag="st")
            p2 = tmp_pool.tile([ROPE, TC], F32, tag="tmp")
            p2s = tmp_pool.tile([ROPE, TC], F32, tag="tmp2")
            nc.vector.tensor_mul(st[:], rope_ps[:], cos_t[:ROPE, :])
            nc.vector.tensor_mul(p2[:], rope_ps[:], sin_t[:ROPE, :])
            nc.vector.tensor_copy(p2s[0:32, :], p2[32:64, :])
            nc.scalar.copy(p2s[32:64, :], p2[0:32, :])
            nc.vector.tensor_add(st[:], st[:], p2s[:])
            dma(koutT[KV_LORA : KV_LORA + ROPE, :], st[:])

            # ---- steps 2+3 per head; rope pairs interleaved
            for g in range(NPAIR):
                for hl in range(2):
                    h = 2 * g + hl
                    w2t = w2_pool.tile([P, TQ, P], F32R, tag="w2")
                    dma(w2t[:], w2n[h])
                    psn = ps23_pool.tile([P, TC], F32, tag="ps23")
                    for j in range(TQ):
                        nc.tensor.matmul(
                            psn[:], w2t[:, j, :], qkv_tiles[j][:],
                            start=(j == 0), stop=(j == TQ - 1),
                        )
                    qn = qn_pool.tile([P, TC], F32R, tag="qn")
                    nc.scalar.copy(qn[:], psn[:])
                    wkct = wkc_pool.tile([P, KV_LORA], F32R, tag="wkc")
                    dma(wkct[:], wkc[h])
                    for j2 in range(TKV):
                        pso = ps23_pool.tile([P, TC], F32, tag="ps23")
                        nc.tensor.matmul(
                            pso[:], wkct[:, j2 * P : (j2 + 1) * P], qn[:],
                            start=True, stop=True,
                        )
                        ost = st_pool.tile([P, TC], F32, tag="st")
                        if j2 % 2 == 0:
                            nc.vector.tensor_copy(ost[:], pso[:])
                        else:
                            nc.scalar.copy(ost[:], pso[:])
                        dma(qoutT[h, j2 * P : (j2 + 1) * P, :], ost[:])

                # rope for the pe parts of heads (2g, 2g+1), packed [128, TC]
                w2t = w2_pool.tile([P, TQ, P], F32R, tag="w2")
                dma(w2t[:], w2p[g])
                psp = ps23_pool.tile([P, TC], F32, tag="ps23")
                for j in range(TQ):
                    nc.tensor.matmul(
                        psp[:], w2t[:, j, :], qkv_tiles[j][:],
                        start=(j == 0), stop=(j == TQ - 1),
                    )
                st = st_pool.tile([P, TC], F32, tag="st")
                p2 = tmp_pool.tile([P, TC], F32, tag="tmp")
                p2s = tmp_pool.tile([P, TC], F32, tag="tmp2")
                nc.vector.tensor_mul(st[:], psp[:], cos_t[:])
                nc.vector.tensor_mul(p2[:], psp[:], sin_t[:])
                for hl in range(2):
                    b = hl * 64
                    nc.vector.tensor_copy(p2s[b : b + 32, :], p2[b + 32 : b + 64, :])
                    nc.scalar.copy(p2s[b + 32 : b + 64, :], p2[b : b + 32, :])
                nc.vector.tensor_add(st[:], st[:], p2s[:])
                for hl in range(2):
                    b = hl * 64
                    dma(
                        qoutT[2 * g + hl, KV_LORA : KV_LORA + ROPE, :],
                        st[b : b + 64, :],
                    )

    nc.compile()
    return nc


# ---------------------------------------------------------------- host prep
def _prep_shared(w_qkv_a, q_a_ln_w, w_q_b, kv_a_ln_w, w_kc):
    """Weight blocking/permutation shared by all cores."""
    # fold the rope de-interleave permutation into the w_qkv_a columns
    wq = np.array(w_qkv_a, dtype=np.float32, copy=True)
    base = Q_LORA + KV_LORA
    perm = list(range(base, ROPE_END, 2)) + list(range(base + 1, ROPE_END, 2))
    wq[:, base:] = np.asarray(w_qkv_a)[:, perm]

    W1 = wq.reshape(KQ, KJ, P, ROPE_END)                        # (kq, kj, ki, m)
    w1f = np.ascontiguousarray(
        W1[:, :, :, : 16 * P].reshape(KQ, KJ, P, 16, P).transpose(3, 0, 2, 1, 4)
    )                                                           # [mt, kq, ki, kj, mi]
    w1r = np.ascontiguousarray(W1[:, :, :, 16 * P :].transpose(0, 2, 1, 3))

    W2 = np.asarray(w_q_b, dtype=np.float32).reshape(TQ, P, NH * QK)  # (kj, ki, m)
    w2n = np.empty((NH, P, TQ, P), np.float32)
    for h in range(NH):
        w2n[h] = W2[:, :, QK * h : QK * h + NOPE].transpose(1, 0, 2)
    w2p = np.empty((NPAIR, P, TQ, P), np.float32)
    for g in range(NPAIR):
        cols = []
        for hl in (0, 1):
            cb = QK * (2 * g + hl) + NOPE
            cols += list(range(cb, cb + ROPE, 2)) + list(range(cb + 1, cb + ROPE, 2))
        w2p[g] = W2[:, :, cols].transpose(1, 0, 2)

    return {
        "ones_d": np.ones((P, 1), np.float32),
        "onesm_d": np.ones((1, P), np.float32),
        "w1f": _round_f32r(w1f),
        "w1r": _round_f32r(w1r),
        "w2n": _round_f32r(w2n),
        "w2p": _round_f32r(w2p),
        "wkc": _round_f32r(np.ascontiguousarray(w_kc, dtype=np.float32)),
        "lnq": np.ascontiguousarray(np.asarray(q_a_ln_w, np.float32).reshape(TQ, P).T),
        "lnkv": np.ascontiguousarray(
            np.asarray(kv_a_ln_w, np.float32).reshape(TKV, P).T
        ),
    }


def _prep_core(hidden, positions, cos_sin_cache, c):
    ht = np.asarray(hidden[c * TC : (c + 1) * TC], dtype=np.float32).T  # [H, TC]
    hid = np.ascontiguousarray(
        ht.reshape(KQ, KJ, P, TC).transpose(0, 2, 1, 3)
    )                                                                   # [kq, ki, kj, t]
    cs = np.asarray(cos_sin_cache, np.float32)[
        np.asarray(positions[c * TC : (c + 1) * TC], np.int64)
    ]                                                                   # [TC, ROPE]
    cosH = np.ascontiguousarray(cs[:, : ROPE // 2].T)                   # [32, TC]
    sinH = np.ascontiguousarray(cs[:, ROPE // 2 :].T)
    # sin carries alternating block signs so rope reduces to P1 + swap(P2)
    return {
        "hid": _round_f32r(hid),
        "cos4": np.ascontiguousarray(np.concatenate([cosH] * 4, 0)),
        "sin4": np.ascontiguousarray(np.concatenate([sinH, -sinH, sinH, -sinH], 0)),
    }


_NC_CACHE = {}


def _get_program(loop_n=None):
    if loop_n not in _NC_CACHE:
        _NC_CACHE[loop_n] = build_program(loop_n)
    return _NC_CACHE[loop_n]


def kernel(
    hidden_states,
    positions,
    w_qkv_a,
    q_a_ln_w,
    w_q_b,
    kv_a_ln_w,
    w_kc,
    cos_sin_cache,
    k_cache,
    rope_cache,
    slot_mapping,
    _trace=False,
):
    from concourse.bass_utils import run_bass_kernel_spmd

    nc = _get_program()
    shared = _prep_shared(w_qkv_a, q_a_ln_w, w_q_b, kv_a_ln_w, w_kc)
    in_maps = []
    for c in range(NCORES):
        m = dict(shared)
        m.update(_prep_core(hidden_states, positions, cos_sin_cache, c))
        in_maps.append(m)

    res = run_bass_kernel_spmd(nc, in_maps, list(range(NCORES)), trace=_trace)

    q_out = np.empty((T, NH, KV_LORA + ROPE), np.float32)
    k_out = np.empty((T, KV_LORA + ROPE), np.float32)
    for c in range(NCORES):
        q_out[c * TC : (c + 1) * TC] = res.results[c]["qoutT"].transpose(2, 0, 1)
        k_out[c * TC : (c + 1) * TC] = res.results[c]["koutT"].T

    slots = np.asarray(slot_mapping, np.int64)
    k_cache_out = np.array(k_cache, dtype=np.float32, copy=True)
    rope_cache_out = np.array(rope_cache, dtype=np.float32, copy=True)
    k_cache_out[slots] = k_out[:, :KV_LORA]
    rope_cache_out[slots] = k_out[:, KV_LORA:]

    if _trace:
        kernel._last_results = res
    return q_out, k_out, k_cache_out, rope_cache_out


# revision 26
# speedup vs baseline: 1.0688x; 1.0688x over previous
"""Fused MLA preprocess kernel for Trainium2, 8 NeuronCores, token-data-parallel.

Layout strategy: every on-device tensor is kept "transposed" (feature on the
SBUF partition dim, tokens on the free dim) so that all three matmuls contract
over the partition dim with weights in their natural layout and zero on-device
transposes.  The host does the pure data-movement parts: sharding tokens,
pre-transposing/blocking inputs, re-transposing outputs, and the cache scatter.

Matmul dtype is float32r (fp32 with ~12-bit mantissa, full PE rate); inputs are
pre-rounded on the host so HW and any simulation agree bit-wise.
"""

import contextlib
import os
import sys

sys.path.insert(0, "/opt/trn_rl_repo")
os.environ.setdefault("MYCRO_LOCAL_CACHE", "1")

import numpy as np

import concourse.bacc as bacc
import concourse.bass as bass
import concourse.mybir as mybir
import concourse.tile as tile

# ---------------------------------------------------------------- constants
T, H = 4096, 7168
Q_LORA, KV_LORA = 1536, 512
ROPE, NOPE, NH = 64, 128, 16
QK = NOPE + ROPE          # 192
SLOTS = 16384
EPS = 1e-6
ROPE_END = Q_LORA + KV_LORA + ROPE   # 2112

NCORES = 8
TC = T // NCORES          # 512 tokens per core
P = 128
KO1 = H // P              # 56 k-tiles for the qkv_a matmul
KQ = 4                    # hidden split into 4 quarters of 14 k-tiles
KJ = KO1 // KQ            # 14
TQ = Q_LORA // P          # 12 q-lora k-tiles
TKV = KV_LORA // P        # 4
M1 = 17                   # step-1 m-tiles: 12 q + 4 latent + 1 rope(64)
NPAIR = NH // 2           # 8 packed rope pairs

F32 = mybir.dt.float32
F32R = mybir.dt.float32r

AluOp = mybir.AluOpType
Act = mybir.ActivationFunctionType


def _round_f32r(x: np.ndarray) -> np.ndarray:
    """Round-to-nearest fp32 -> fp32r (12 low mantissa bits cleared)."""
    u = np.ascontiguousarray(x, dtype=np.float32).view(np.uint32)
    r = (u + np.uint32(0x7FF) + ((u >> np.uint32(12)) & np.uint32(1))) & np.uint32(
        0xFFFFF000
    )
    return r.view(np.float32)


# ---------------------------------------------------------------- program
def build_program(loop_n: int | None = None) -> bass.Bass:
    nc = bacc.Bacc("TRN2", target_bir_lowering=False, debug=False)

    # Round-robin DMA issue between the two HWDGE-capable engines (SP, ACT)
    # so neither sequencer's per-dma_start issue overhead serializes the
    # kernel.  Weight/activation loads lean on SP; ACT takes every third.
    _dma_ctr = [0]

    def dma(out_ap, in_ap):
        _dma_ctr[0] += 1
        eng = nc.scalar if (_dma_ctr[0] % 2 == 0) else nc.sync
        return eng.dma_start(out_ap, in_ap)

    hid = nc.dram_tensor("hid", (KQ, P, KJ, TC), F32R, kind="ExternalInput")
    w1f = nc.dram_tensor("w1f", (16, KQ, P, KJ, P), F32R, kind="ExternalInput")
    w1r = nc.dram_tensor("w1r", (KQ, P, KJ, ROPE), F32R, kind="ExternalInput")
    w2n = nc.dram_tensor("w2n", (NH, P, TQ, P), F32R, kind="ExternalInput")
    w2p = nc.dram_tensor("w2p", (NPAIR, P, TQ, P), F32R, kind="ExternalInput")
    wkc = nc.dram_tensor("wkc", (NH, P, KV_LORA), F32R, kind="ExternalInput")
    lnq = nc.dram_tensor("lnq", (P, TQ), F32, kind="ExternalInput")
    lnkv = nc.dram_tensor("lnkv", (P, TKV), F32, kind="ExternalInput")
    cos4 = nc.dram_tensor("cos4", (P, TC), F32, kind="ExternalInput")
    sin4 = nc.dram_tensor("sin4", (P, TC), F32, kind="ExternalInput")
    ones_d = nc.dram_tensor("ones_d", (P, 1), F32R, kind="ExternalInput")
    onesm_d = nc.dram_tensor("onesm_d", (1, P), F32R, kind="ExternalInput")
    qoutT = nc.dram_tensor("qoutT", (NH, KV_LORA + ROPE, TC), F32, kind="ExternalOutput")
    koutT = nc.dram_tensor("koutT", (KV_LORA + ROPE, TC), F32, kind="ExternalOutput")

    with tile.TileContext(nc) as tc:
        with (
            tc.tile_pool(name="hid_sb", bufs=1) as hid_pool,
            tc.tile_pool(name="w1_sb", bufs=2) as w1_pool,
            tc.tile_pool(name="qkv_sb", bufs=1) as qkv_pool,
            tc.tile_pool(name="sq_sb", bufs=1) as sq_pool,
            tc.tile_pool(name="w2_sb", bufs=2) as w2_pool,
            tc.tile_pool(name="wkc_sb", bufs=2) as wkc_pool,
            tc.tile_pool(name="const_sb", bufs=1) as const_pool,
            tc.tile_pool(name="qn_sb", bufs=2) as qn_pool,
            tc.tile_pool(name="st_sb", bufs=4) as st_pool,
            tc.tile_pool(name="tmp_sb", bufs=1) as tmp_pool,
            tc.tile_pool(name="ps1", bufs=2, space="PSUM") as ps1_pool,
            tc.tile_pool(name="pss", bufs=1, space="PSUM") as pss_pool,
            tc.tile_pool(name="psb", bufs=1, space="PSUM") as psb_pool,
            tc.tile_pool(name="ps23", bufs=3, space="PSUM") as ps23_pool,
            (
                tc.For_i(0, loop_n, 1)
                if loop_n is not None
                else contextlib.nullcontext()
            ),
        ):
            # ---- constants / small inputs
            lnq_t = const_pool.tile([P, TQ], F32, tag="lnq")
            lnkv_t = const_pool.tile([P, TKV], F32, tag="lnkv")
            cos_t = const_pool.tile([P, TC], F32, tag="cos4")
            sin_t = const_pool.tile([P, TC], F32, tag="sin4")
            ones_k = const_pool.tile([P, 1], F32R, tag="ones_k")   # lhsT for column sums
            ones_m = const_pool.tile([1, P], F32R, tag="ones_m")   # lhsT for broadcast
            s_q = const_pool.tile([1, TC], F32R, tag="s_q")
            s_kv = const_pool.tile([1, TC], F32R, tag="s_q")   # sequential reuse
            ms_q = const_pool.tile([1, TC], F32, tag="ms_q")
            ms_kv = const_pool.tile([1, TC], F32, tag="ms_q")  # sequential reuse
            dma(lnq_t[:], lnq[:])
            dma(lnkv_t[:], lnkv[:])
            dma(cos_t[:], cos4[:])
            dma(sin_t[:], sin4[:])
            dma(ones_k[:], ones_d[:])
            dma(ones_m[:], onesm_d[:])

            # ---- resident hidden (transposed, blocked by k-quarter)
            hid_tiles = []
            for kq in range(KQ):
                ht = hid_pool.tile([P, KJ, TC], F32R, tag=f"hid{kq}")
                dma(ht[:], hid[kq])
                hid_tiles.append(ht)

            # ---- step 1: qkvT = w_qkv_a.T @ hiddenT   (17 m-tiles)
            qkv_tiles = []          # 12 q (f32r) + 4 latent (f32); rope stays in PSUM
            sumq_ps = pss_pool.tile([1, TC], F32, tag="sums")
            sumkv_ps = None
            sq_bq = None
            sq_bkv = None
            rope_ps = None

            for mt in range(M1):
                msz = P if mt < 16 else ROPE
                ps = ps1_pool.tile([msz, TC], F32, tag="ps1")
                for kq in range(KQ):
                    wt = w1_pool.tile([P, KJ, msz], F32R, tag="w1")
                    if mt < 16:
                        dma(wt[:], w1f[mt, kq])
                    else:
                        dma(wt[:], w1r[kq])
                    for kj in range(KJ):
                        nc.tensor.matmul(
                            ps[:],
                            wt[:, kj, :],
                            hid_tiles[kq][:, kj, :],
                            start=(kq == 0 and kj == 0),
                            stop=(kq == KQ - 1 and kj == KJ - 1),
                        )

                if mt < TQ:
                    # q-lora tile: raw copyback (f32r) + squares for rmsnorm
                    qt = qkv_pool.tile([P, TC], F32R, tag=f"qkv{mt}")
                    nc.vector.tensor_copy(qt[:], ps[:])
                    sq = sq_pool.tile([P, TC], F32R, tag="sq")
                    nc.scalar.activation(sq[:], ps[:], Act.Square)
                    nc.tensor.matmul(
                        sumq_ps[:], ones_k[:], sq[:],
                        start=(mt == 0), stop=(mt == TQ - 1),
                    )
                    qkv_tiles.append(qt)
                elif mt < 16:
                    # kv-latent tile: raw copyback (f32) + squares
                    qt = qkv_pool.tile([P, TC], F32, tag=f"qkv{mt}")
                    nc.vector.tensor_copy(qt[:], ps[:])
                    sq = sq_pool.tile([P, TC], F32R, tag="sq")
                    nc.scalar.activation(sq[:], ps[:], Act.Square)
                    if mt == TQ:
                        sumkv_ps = pss_pool.tile([1, TC], F32, tag="sums")
                    nc.tensor.matmul(
                        sumkv_ps[:], ones_k[:], sq[:],
                        start=(mt == TQ), stop=(mt == 15),
                    )
                    qkv_tiles.append(qt)
                else:
                    rope_ps = ps  # kept in PSUM for the k-rope below

                if mt == TQ - 1:
                    # s_q = 1/sqrt(mean + eps), broadcast to 128 partitions
                    nc.scalar.activation(
                        ms_q[:], sumq_ps[:], Act.Copy, scale=1.0 / Q_LORA, bias=EPS
                    )
                    nc.vector.reciprocal(ms_q[:], ms_q[:])
                    nc.scalar.activation(s_q[:], ms_q[:], Act.Sqrt)
                    sq_bq = psb_pool.tile([P, TC], F32, tag="bq")
                    nc.tensor.matmul(sq_bq[:], ones_m[:], s_q[:], start=True, stop=True)
                    # scale all 12 q tiles in place: q = (q * lnw) * s
                    for j in range(TQ):
                        nc.vector.scalar_tensor_tensor(
                            qkv_tiles[j][:], qkv_tiles[j][:], lnq_t[:, j : j + 1],
                            sq_bq[:], AluOp.mult, AluOp.mult,
                        )
                if mt == 15:
                    nc.scalar.activation(
                        ms_kv[:], sumkv_ps[:], Act.Copy, scale=1.0 / KV_LORA, bias=EPS
                    )
                    nc.vector.reciprocal(ms_kv[:], ms_kv[:])
                    nc.scalar.activation(s_kv[:], ms_kv[:], Act.Sqrt)
                    sq_bkv = psb_pool.tile([P, TC], F32, tag="bkv")
                    nc.tensor.matmul(sq_bkv[:], ones_m[:], s_kv[:], start=True, stop=True)
                    # k_nope tiles -> staging -> DRAM
                    for j in range(TKV):
                        st = st_pool.tile([P, TC], F32, tag="st")
                        nc.vector.scalar_tensor_tensor(
                            st[:], qkv_tiles[TQ + j][:], lnkv_t[:, j : j + 1],
                            sq_bkv[:], AluOp.mult, AluOp.mult,
                        )
                        dma(koutT[j * P : (j + 1) * P, :], st[:])

            # ---- k-rope (rows live in rope_ps [64, TC]).  sin_t carries
            # alternating signs per 32-block, so after a partition-swap of the
            # halves the result is just P1 + swap(P2):
            #   out[0:32] = x1*cos - x2*sin;  out[32:64] = x2*cos + x1*sin
            st = st_pool.tile([ROPE, TC], F32, tag="st")
            p2 = tmp_pool.tile([ROPE, TC], F32, tag="tmp")
            p2s = tmp_pool.tile([ROPE, TC], F32, tag="tmp2")
            nc.vector.tensor_mul(st[:], rope_ps[:], cos_t[:ROPE, :])
            nc.vector.tensor_mul(p2[:], rope_ps[:], sin_t[:ROPE, :])
            nc.vector.tensor_copy(p2s[0:32, :], p2[32:64, :])
            nc.scalar.copy(p2s[32:64, :], p2[0:32, :])
            nc.vector.tensor_add(st[:], st[:], p2s[:])
            dma(koutT[KV_LORA : KV_LORA + ROPE, :], st[:])

            # ---- steps 2+3 per head; rope pairs interleaved
            for g in range(NPAIR):
                for hl in range(2):
                    h = 2 * g + hl
                    w2t = w2_pool.tile([P, TQ, P], F32R, tag="w2")
                    dma(w2t[:], w2n[h])
                    psn = ps23_pool.tile([P, TC], F32, tag="ps23")
                    for j in range(TQ):
                        nc.tensor.matmul(
                            psn[:], w2t[:, j, :], qkv_tiles[j][:],
                            start=(j == 0), stop=(j == TQ - 1),
                        )
                    qn = qn_pool.tile([P, TC], F32R, tag="qn")
                    nc.scalar.copy(qn[:], psn[:])
                    wkct = wkc_pool.tile([P, KV_LORA], F32R, tag="wkc")
                    dma(wkct[:], wkc[h])
                    for j2 in range(TKV):
                        pso = ps23_pool.tile([P, TC], F32, tag="ps23")
                        nc.tensor.matmul(
                            pso[:], wkct[:, j2 * P : (j2 + 1) * P], qn[:],
                            start=True, stop=True,
                        )
                        ost = st_pool.tile([P, TC], F32, tag="st")
                        if j2 % 2 == 0:
                            nc.vector.tensor_copy(ost[:], pso[:])
                        else:
                            nc.scalar.copy(ost[:], pso[:])
                        dma(qoutT[h, j2 * P : (j2 + 1) * P, :], ost[:])

                # rope for the pe parts of heads (2g, 2g+1), packed [128, TC]
                w2t = w2_pool.tile([P, TQ, P], F32R, tag="w2")
                dma(w2t[:], w2p[g])
                psp = ps23_pool.tile([P, TC], F32, tag="ps23")
                for j in range(TQ):
                    nc.tensor.matmul(
                        psp[:], w2t[:, j, :], qkv_tiles[j][:],
                        start=(j == 0), stop=(j == TQ - 1),
                    )
                st = st_pool.tile([P, TC], F32, tag="st")
                p2 = tmp_pool.tile([P, TC], F32, tag="tmp")
                p2s = tmp_pool.tile([P, TC], F32, tag="tmp2")
                nc.vector.tensor_mul(st[:], psp[:], cos_t[:])
                nc.vector.tensor_mul(p2[:], psp[:], sin_t[:])
                for hl in range(2):
                    b = hl * 64
                    nc.vector.tensor_copy(p2s[b : b + 32, :], p2[b + 32 : b + 64, :])
                    nc.scalar.copy(p2s[b + 32 : b + 64, :], p2[b : b + 32, :])
                nc.vector.tensor_add(st[:], st[:], p2s[:])
                for hl in range(2):
                    b = hl * 64
                    dma(
                        qoutT[2 * g + hl, KV_LORA : KV_LORA + ROPE, :],
                        st[b : b + 64, :],
                    )

    nc.compile()
    return nc


# ---------------------------------------------------------------- host prep
def _prep_shared(w_qkv_a, q_a_ln_w, w_q_b, kv_a_ln_w, w_kc):
    """Weight blocking/permutation shared by all cores."""
    # fold the rope de-interleave permutation into the w_qkv_a columns
    wq = np.array(w_qkv_a, dtype=np.float32, copy=True)
    base = Q_LORA + KV_LORA
    perm = list(range(base, ROPE_END, 2)) + list(range(base + 1, ROPE_END, 2))
    wq[:, base:] = np.asarray(w_qkv_a)[:, perm]

    W1 = wq.reshape(KQ, KJ, P, ROPE_END)                        # (kq, kj, ki, m)
    w1f = np.ascontiguousarray(
        W1[:, :, :, : 16 * P].reshape(KQ, KJ, P, 16, P).transpose(3, 0, 2, 1, 4)
    )                                                           # [mt, kq, ki, kj, mi]
    w1r = np.ascontiguousarray(W1[:, :, :, 16 * P :].transpose(0, 2, 1, 3))

    W2 = np.asarray(w_q_b, dtype=np.float32).reshape(TQ, P, NH * QK)  # (kj, ki, m)
    w2n = np.empty((NH, P, TQ, P), np.float32)
    for h in range(NH):
        w2n[h] = W2[:, :, QK * h : QK * h + NOPE].transpose(1, 0, 2)
    w2p = np.empty((NPAIR, P, TQ, P), np.float32)
    for g in range(NPAIR):
        cols = []
        for hl in (0, 1):
            cb = QK * (2 * g + hl) + NOPE
            cols += list(range(cb, cb + ROPE, 2)) + list(range(cb + 1, cb + ROPE, 2))
        w2p[g] = W2[:, :, cols].transpose(1, 0, 2)

    return {
        "ones_d": np.ones((P, 1), np.float32),
        "onesm_d": np.ones((1, P), np.float32),
        "w1f": _round_f32r(w1f),
        "w1r": _round_f32r(w1r),
        "w2n": _round_f32r(w2n),
        "w2p": _round_f32r(w2p),
        "wkc": _round_f32r(np.ascontiguousarray(w_kc, dtype=np.float32)),
        "lnq": np.ascontiguousarray(np.asarray(q_a_ln_w, np.float32).reshape(TQ, P).T),
        "lnkv": np.ascontiguousarray(
            np.asarray(kv_a_ln_w, np.float32).reshape(TKV, P).T
        ),
    }


def _prep_core(hidden, positions, cos_sin_cache, c):
    ht = np.asarray(hidden[c * TC : (c + 1) * TC], dtype=np.float32).T  # [H, TC]
    hid = np.ascontiguousarray(
        ht.reshape(KQ, KJ, P, TC).transpose(0, 2, 1, 3)
    )                                                                   # [kq, ki, kj, t]
    cs = np.asarray(cos_sin_cache, np.float32)[
        np.asarray(positions[c * TC : (c + 1) * TC], np.int64)
    ]                                                                   # [TC, ROPE]
    cosH = np.ascontiguousarray(cs[:, : ROPE // 2].T)                   # [32, TC]
    sinH = np.ascontiguousarray(cs[:, ROPE // 2 :].T)
    # sin carries alternating block signs so rope reduces to P1 + swap(P2)
    return {
        "hid": _round_f32r(hid),
        "cos4": np.ascontiguousarray(np.concatenate([cosH] * 4, 0)),
        "sin4": np.ascontiguousarray(np.concatenate([sinH, -sinH, sinH, -sinH], 0)),
    }


_NC_CACHE = {}


def _get_program(loop_n=None):
    if loop_n not in _NC_CACHE:
        _NC_CACHE[loop_n] = build_program(loop_n)
    return _NC_CACHE[loop_n]


def kernel(
    hidden_states,
    positions,
    w_qkv_a,
    q_a_ln_w,
    w_q_b,
    kv_a_ln_w,
    w_kc,
    cos_sin_cache,
    k_cache,
    rope_cache,
    slot_mapping,
    _trace=False,
):
    from concourse.bass_utils import run_bass_kernel_spmd

    nc = _get_program()
    shared = _prep_shared(w_qkv_a, q_a_ln_w, w_q_b, kv_a_ln_w, w_kc)
    in_maps = []
    for c in range(NCORES):
        m = dict(shared)
        m.update(_prep_core(hidden_states, positions, cos_sin_cache, c))
        in_maps.append(m)

    res = run_bass_kernel_spmd(nc, in_maps, list(range(NCORES)), trace=_trace)

    q_out = np.empty((T, NH, KV_LORA + ROPE), np.float32)
    k_out = np.empty((T, KV_LORA + ROPE), np.float32)
    for c in range(NCORES):
        q_out[c * TC : (c + 1) * TC] = res.results[c]["qoutT"].transpose(2, 0, 1)
        k_out[c * TC : (c + 1) * TC] = res.results[c]["koutT"].T

    slots = np.asarray(slot_mapping, np.int64)
    k_cache_out = np.array(k_cache, dtype=np.float32, copy=True)
    rope_cache_out = np.array(rope_cache, dtype=np.float32, copy=True)
    k_cache_out[slots] = k_out[:, :KV_LORA]
    rope_cache_out[slots] = k_out[:, KV_LORA:]

    if _trace:
        kernel._last_results = res
    return q_out, k_out, k_cache_out, rope_cache_out
